# revision 12
# baseline (speedup 1.0000x reference)
"""Trainium2 Bass kernel for nn_ContrastLoss_Disentangle.

Contract: kernel(**inputs) takes the FULL (unsharded) inputs and returns the
same structure the reference returns: (loss_label, loss_norm, loss_triple)
as float32 scalars.

Pipeline (8 NeuronCores, data-parallel, fp16 device inputs):
  host:    norms (f32, exact), normalization, cat-folding y = nlpF*cat
           (so the device dot IS the score), poseFT panels (fp16)
  device1: per-core: scores via DVE tensor_tensor_reduce (fused mult+
           reduce over the 2048-long (c,d) axis), plus a symmetric-blocked
           slice of the pose gram matrix pm on the PE (fp16 matmul):
           every core computes O = L^T R (one full [512,512] off-diag
           block) and D = R^T R (upper-trimmed diagonal block); 6 of the
           8 (L,R) assignments produce the 6 off-diag + 4 diag blocks.
  host:    BCE from scores, pm assembly (mirror the triangle) + stable
           argsort rank-select (furthest), gather of hard-positive y rows
  device2: per-core fused dot-reduces of the gathered rows -> cur
  host:    triplet loss assembly
"""

import os
import numpy as np

import concourse.bass as bass
import concourse.tile as tile
from concourse import bacc, mybir
from concourse.bass2jax import install_neuronx_cc_hook, partition_id_tensor, _bass_exec_p

C, NP, K, D = 8, 2048, 4, 256
NN = NP * K          # 8192
NCORES = 8
NPL = NP // NCORES   # 256 poses per core
CD = C * D           # 2048 contraction size
KCD = K * CD         # 8192 per-pose y row length
S = 512              # pm strip width
KT = CD // 128       # 16 contraction chunks
NSTRIP = NP // S     # 4 strips

# (L, R, rot) strip assignment per core: O = L^T R covers the 6 off-diagonal
# [512,512] blocks (cores 6,7 duplicate); D = R^T R m-chunks 0,1 cover the
# TOP half of diagonal block R on cores 0-3 and -- via a 256-column rotation
# of the R panel (rot=1) -- the BOTTOM half on cores 4-7.
PM_ASSIGN = [(1, 0, 0), (2, 1, 0), (3, 2, 0), (0, 3, 0),
             (0, 2, 1), (1, 3, 1), (2, 0, 1), (3, 1, 1)]
ROT = 256

F16 = mybir.dt.float16

_runners = {}


def _build_dots_kernel(with_pm: bool, with_ssq: bool = True):
    """Per-core program. Inputs (per core):
      y     [NPL, KCD] f16  (cat-folded normalized nlp rows, [k,c,d] inner)
      pose  [NPL, CD] f16   (normalized pose rows, [c,d] inner)
      pml / pmr [CD, S] f16 (poseFT strip panels; only when with_pm)
    Outputs:
      scores [128, 2K] f32  (col = half*K + k; partition p -> pose
                             128*half + p of this core's shard)
      pmo [S, S] f16        L^T R block (only when with_pm)
      pmd [256, S] f16      R^T R rows 0:128 (all cols) and 128:256
                            (cols 128:512) -- the top half of the diag
                            gram (bottom half comes from rotated panels)
    """
    del with_ssq  # kept for test.py signature compatibility
    nc = bacc.Bacc("TRN2", target_bir_lowering=False, debug=False,
                   num_devices=NCORES)
    y = nc.dram_tensor("y", [NPL, KCD], F16, kind="ExternalInput").ap()
    pose = nc.dram_tensor("pose", [NPL, CD], F16, kind="ExternalInput").ap()
    if with_pm:
        pml = nc.dram_tensor("pml", [CD, S], F16, kind="ExternalInput").ap()
        pmr = nc.dram_tensor("pmr", [CD, S], F16, kind="ExternalInput").ap()
        pmo = nc.dram_tensor("pmo", [S, S], F16, kind="ExternalOutput").ap()
        pmd = nc.dram_tensor("pmd", [256, S], F16, kind="ExternalOutput").ap()
    scores = nc.dram_tensor("scores", [128, 2 * K], mybir.dt.float32,
                            kind="ExternalOutput").ap()
    scores_d = nc.dram_tensor("scores_d", [128, 2], mybir.dt.float32,
                              kind="ExternalOutput").ap()

    with tile.TileContext(nc) as tc:
        with tc.tile_pool(name="io", bufs=1) as io, \
             tc.tile_pool(name="scr", bufs=1) as scr, \
             tc.tile_pool(name="accum", bufs=1) as accum, \
             tc.tile_pool(name="mat", bufs=1) as mat, \
             tc.tile_pool(name="ev", bufs=1) as ev, \
             tc.tile_pool(name="ps", bufs=1, space="PSUM") as ps:

            sc = accum.tile([128, 2 * K], mybir.dt.float32, tag="sc", name="sc")
            sc_d = accum.tile([128, 2], mybir.dt.float32, tag="sc_d",
                              name="sc_d")

            def load_pose(h):
                po = io.tile([128, CD], F16, tag=f"po{h}", name=f"po{h}")
                nc.sync.dma_start(po[:], pose[128 * h:128 * (h + 1), :])
                return po

            def load_y_slice(h, k):
                yk = io.tile([128, CD], F16, tag=f"yk{h}{k}", name=f"yk{h}{k}")
                nc.sync.dma_start(
                    yk[:], y[128 * h:128 * (h + 1), CD * k:CD * (k + 1)])
                return yk

            def dots_k(h, k, po, yk):
                # z = y_k * pose (DVE, fp16 2x); sum_x z -> scores col
                # (ACT Copy+accum for most k, DVE tensor_reduce for k==3
                #  to balance the two engines)
                z = scr.tile([128, CD], F16, tag=f"z{h}{k}", name=f"z{h}{k}")
                nc.vector.tensor_tensor(z[:], yk[:], po[:],
                                        op=mybir.AluOpType.mult)
                if k == 0:
                    # DVE-reduced cols live in their own accum tile so the
                    # ACT reductions never serialize against them
                    col = sc_d[:, h:h + 1]
                    nc.vector.tensor_reduce(
                        col, z[:].rearrange("p (o x) -> p o x", o=1),
                        axis=mybir.AxisListType.X, op=mybir.AluOpType.add)
                else:
                    col = sc[:, h * K + k:h * K + k + 1]
                    s = scr.tile([128, CD], F16, tag=f"s{h}{k}", name=f"s{h}{k}")
                    nc.scalar.activation(
                        s[:], z[:], mybir.ActivationFunctionType.Copy,
                        accum_out=col)

            if with_pm:
                lt = mat.tile([128, KT, S], F16, tag="lt", name="lt")
                rt = mat.tile([128, KT, S], F16, tag="rt", name="rt")

                acc_o = [ps.tile([128, S], mybir.dt.float32, tag=f"ao{m}",
                                 name=f"ao{m}") for m in range(4)]
                acc_d = [ps.tile([128, S if m == 0 else S - 128],
                                 mybir.dt.float32, tag=f"ad{m}",
                                 name=f"ad{m}") for m in range(2)]

                def pm_k(k):
                    st, sp = (k == 0), (k == KT - 1)
                    for m in range(4):
                        nc.tensor.matmul(acc_o[m][:],
                                         lt[:, k, 128 * m:128 * (m + 1)],
                                         rt[:, k, :], start=st, stop=sp)
                    # diag top half: rows 0:128 x all cols, rows 128:256 x
                    # cols 128:512 (bottom half comes from rotated panels
                    # on cores 4-7)
                    nc.tensor.matmul(acc_d[0][:],
                                     rt[:, k, 0:128], rt[:, k, :],
                                     start=st, stop=sp)
                    nc.tensor.matmul(acc_d[1][:],
                                     rt[:, k, 128:256], rt[:, k, 128:S],
                                     start=st, stop=sp)

                # panel chunks lead, issued round-robin from the SP / ACT /
                # Pool queues so issue-side serialization never starves the
                # PE; y/pose slices fill the remaining DMA slack
                qs = [nc.sync, nc.scalar, nc.gpsimd]
                qi = 0
                def load_panel_chunk_q(t, srcp, kc):
                    nonlocal qi
                    qs[qi % 3].dma_start(
                        t[:, 4 * kc:4 * (kc + 1), :],
                        srcp[512 * kc:512 * (kc + 1), :]
                        .rearrange("(k p) m -> p k m", p=128))
                    qi += 1
                load_panel_chunk_q(lt, pml, 0)
                load_panel_chunk_q(rt, pmr, 0)
                load_panel_chunk_q(lt, pml, 1)
                load_panel_chunk_q(rt, pmr, 1)
                po0 = load_pose(0)
                yks0 = [load_y_slice(0, k) for k in range(2)]
                load_panel_chunk_q(lt, pml, 2)
                load_panel_chunk_q(rt, pmr, 2)
                yks0 += [load_y_slice(0, k) for k in range(2, K)]
                load_panel_chunk_q(lt, pml, 3)
                load_panel_chunk_q(rt, pmr, 3)
                po1 = load_pose(1)
                yks1 = [load_y_slice(1, k) for k in range(K)]
                for k in range(4):
                    pm_k(k)
                for k in range(K):
                    dots_k(0, k, po0, yks0[k])
                for k in range(4, 12):
                    pm_k(k)
                for k in range(K):
                    dots_k(1, k, po1, yks1[k])
                for k in range(12, 16):
                    pm_k(k)

                # evictions: split across ACT and DVE, each engine issues its
                # own writeback DMA so nothing funnels through the SP queue
                for m in range(4):
                    o = ev.tile([128, S], F16, tag=f"evo{m}", name=f"evo{m}")
                    if m % 2 == 0:
                        nc.scalar.copy(o[:], acc_o[m][:])
                        nc.scalar.dma_start(pmo[128 * m:128 * (m + 1), :], o[:])
                    else:
                        nc.vector.tensor_scalar_mul(o[:], acc_o[m][:], 1.0)
                        nc.gpsimd.dma_start(pmo[128 * m:128 * (m + 1), :], o[:])
                o1 = ev.tile([128, S], F16, tag="evd0", name="evd0")
                nc.scalar.copy(o1[:], acc_d[0][:])
                nc.scalar.dma_start(pmd[0:128, :], o1[:])
                o2 = ev.tile([128, S - 128], F16, tag="evd1", name="evd1")
                nc.vector.tensor_scalar_mul(o2[:], acc_d[1][:], 1.0)
                nc.gpsimd.dma_start(pmd[128:256, 128:S], o2[:])
            else:
                po0 = load_pose(0)
                yks0 = [load_y_slice(0, k) for k in range(K)]
                po1 = load_pose(1)
                yks1 = [load_y_slice(1, k) for k in range(K)]
                for k in range(K):
                    dots_k(0, k, po0, yks0[k])
                for k in range(K):
                    dots_k(1, k, po1, yks1[k])

            nc.scalar.dma_start(scores[:], sc[:])
            nc.sync.dma_start(scores_d[:], sc_d[:])

    nc.finalize()
    return nc


def _make_runner(nc):
    """Reusable jitted SPMD runner (replicates bass2jax.run_bass_via_pjrt but
    caches the compiled executable across calls)."""
    import jax
    from jax.sharding import Mesh, PartitionSpec
    from jax.experimental.shard_map import shard_map

    install_neuronx_cc_hook()
    partition_name = nc.partition_id_tensor.name if nc.partition_id_tensor else None
    in_names, out_names, out_avals = [], [], []
    for alloc in nc.m.functions[0].allocations:
        if not isinstance(alloc, mybir.MemoryLocationSet):
            continue
        name = alloc.memorylocations[0].name
        if alloc.kind == "ExternalInput":
            if name != partition_name:
                in_names.append(name)
        elif alloc.kind == "ExternalOutput":
            out_names.append(name)
            out_avals.append(jax.core.ShapedArray(
                tuple(alloc.tensor_shape), mybir.dt.np(alloc.dtype)))
    n_params = len(in_names)
    all_in = in_names + out_names + ([partition_name] if partition_name else [])

    def _body(*args):
        operands = list(args)
        if partition_name is not None:
            operands.append(partition_id_tensor())
        outs = _bass_exec_p.bind(
            *operands, out_avals=tuple(out_avals), in_names=tuple(all_in),
            out_names=tuple(out_names), lowering_input_output_aliases=(),
            sim_require_finite=False, sim_require_nnan=False, nc=nc)
        return tuple(outs)

    devices = jax.devices()[:NCORES]
    mesh = Mesh(np.asarray(devices), ("core",))
    donate = tuple(range(n_params, n_params + len(out_names)))
    sharded = jax.jit(
        shard_map(_body, mesh=mesh,
                  in_specs=(PartitionSpec("core"),) * (n_params + len(out_names)),
                  out_specs=(PartitionSpec("core"),) * len(out_names),
                  check_rep=False),
        donate_argnums=donate, keep_unused=True)

    def run(in_maps):
        concat_in = [np.concatenate([np.asarray(m[name]) for m in in_maps], axis=0)
                     for name in in_names]
        zeros = [np.zeros((NCORES * a.shape[0], *a.shape[1:]), a.dtype)
                 for a in out_avals]
        out_arrs = sharded(*concat_in, *zeros)
        return [
            {name: np.asarray(out_arrs[i]).reshape(NCORES, *out_avals[i].shape)[c]
             for i, name in enumerate(out_names)}
            for c in range(NCORES)
        ]

    return run


def _get_runner(key):
    if key not in _runners:
        if key == "k1":
            _runners[key] = _make_runner(_build_dots_kernel(with_pm=True))
        else:
            _runners[key] = _make_runner(
                _build_dots_kernel(with_pm=False, with_ssq=False))
    return _runners[key]


def _scores_from_out(res):
    """[8 cores]['scores'][128, 2K] + ['scores_d'][128, 2] -> [NN] scores.

    col = h*K + k, partition p -> pose 128h + p of core shard ->
    nlp row (core*NPL + 128h + p)*K + k; k==0 cols come from scores_d."""
    out = np.empty((NP, K), np.float32)
    for c in range(NCORES):
        blk = res[c]["scores"].reshape(128, 2, K).copy()
        blk[:, :, 0] = res[c]["scores_d"]
        out[c * NPL:(c + 1) * NPL] = blk.transpose(1, 0, 2).reshape(NPL, K)
    return out.reshape(NN)


def _kernel_host_fallback(inputs):
    """Pure-numpy reference replication, used only if the index tensors do
    not have the canonical arange structure the device layout relies on."""
    nlp = np.asarray(inputs["nlp_features"], np.float32)
    pose = np.asarray(inputs["pose_features"], np.float32)
    nlab = np.asarray(inputs["nlp_label"]).astype(np.int64)
    n2p = np.asarray(inputs["nlpid2poseid"]).astype(np.int64)
    p2n = np.asarray(inputs["pose2nlpid"]).astype(np.int64)
    cat = np.asarray(inputs["categories"], np.float32)
    ri = np.asarray(inputs["rand_index"]).astype(np.int64)
    Np, Nn = pose.shape[1], nlp.shape[1]
    norm_p = np.sqrt(np.einsum("cpd,cpd->cp", pose, pose, dtype=np.float32))
    norm_n = np.sqrt(np.einsum("cnd,cnd->cn", nlp, nlp, dtype=np.float32))
    poseF = pose / norm_p[:, :, None]
    nlpF = nlp / norm_n[:, :, None]
    loss_norm = np.float32(np.float32(norm_p.mean()) + np.float32(norm_n.mean()))
    dots = np.einsum("cnd,cnd->cn", nlpF, poseF[:, n2p]).astype(np.float32)
    scores = np.einsum("cn,nc->n", dots, cat).astype(np.float32)
    p = (1.0 / (1.0 + np.exp(-scores))).astype(np.float32)
    lblf = nlab.astype(np.float32)
    loss_label = np.float32(
        np.mean(-(np.log(p) * lblf + np.log(1.0 - p) * (1.0 - lblf))))
    pf = np.ascontiguousarray(poseF.transpose(0, 2, 1).reshape(-1, Np))
    pm = (pf.T @ pf).astype(np.float32)
    ar = np.arange(Np)
    pm[ar, ar] = 1.0
    order = np.argsort(pm, axis=1, kind="stable")
    furthest = order[ar, ri]
    sg = scores[p2n]
    lg = nlab[p2n]
    maxp = np.maximum(np.max(np.where(lg == 0, sg, -np.inf), axis=1), -1.0)
    minp = np.minimum(np.min(np.where(lg == 1, sg, np.inf), axis=1), 1.0)
    nids = p2n[furthest]
    cd = np.einsum("cpkd,cpd->cpk", nlpF[:, nids], poseF)
    cur = np.einsum("cpk,pkc->pk", cd, cat[nids]).astype(np.float32)
    lcur = nlab[nids]
    maxcur = np.max(np.where(lcur == 1, cur, -np.inf), axis=1)
    maxp = np.maximum(maxp, maxcur)
    found = ~((maxp == -1.0) | (minp == 1.0))
    lt = np.where(found, maxp - minp + 2.0, 0.0).astype(np.float32)
    nf = int(np.sum(~found))
    loss_triple = (np.float32(0.0) if nf == Nn else
                   np.float32(lt.sum(dtype=np.float32) / np.float32(Nn - nf)))
    return (np.float32(loss_label), loss_norm, loss_triple)


def kernel(**inputs):
    nlp = np.ascontiguousarray(inputs["nlp_features"], np.float32)      # [C, NN, D]
    pose = np.ascontiguousarray(inputs["pose_features"], np.float32)    # [C, NP, D]
    nlab = np.asarray(inputs["nlp_label"]).astype(np.int64)
    cat = np.ascontiguousarray(inputs["categories"], np.float32)        # [NN, C]
    ri = np.asarray(inputs["rand_index"]).astype(np.int64)

    n2p = np.asarray(inputs["nlpid2poseid"]).astype(np.int64)
    p2n = np.asarray(inputs["pose2nlpid"]).astype(np.int64)
    if (not np.array_equal(n2p, np.arange(NN) // K)
            or not np.array_equal(p2n, np.arange(NN).reshape(NP, K))):
        return _kernel_host_fallback(inputs)

    # ---- host: exact norms (f32), normalization, cat-folded y ------------
    norm_p = np.sqrt(np.einsum("cpd,cpd->cp", pose, pose, dtype=np.float32,
                               optimize=True)).astype(np.float32)       # [C, NP]
    norm_n = np.sqrt(np.einsum("cnd,cnd->cn", nlp, nlp, dtype=np.float32,
                               optimize=True)).astype(np.float32)       # [C, NN]
    loss_norm = np.float32(np.float32(norm_p.mean()) + np.float32(norm_n.mean()))

    poseF = pose / norm_p[:, :, None]                                   # [C, NP, D]
    nlpF = nlp / norm_n[:, :, None]                                     # [C, NN, D]
    # y[n, c, d] = nlpF[c, n, d] * cat[n, c]; device layout [NP, K, C, D]
    y = (nlpF.transpose(1, 0, 2) * cat[:, :, None]).astype(np.float16)
    y = np.ascontiguousarray(y.reshape(NP, K, CD).reshape(NP, KCD))
    # pose rows in [p][c,d] layout
    pose_dev = np.ascontiguousarray(
        poseF.transpose(1, 0, 2).reshape(NP, CD)).astype(np.float16)
    # poseFT strip panels for the gram
    poseFT = np.ascontiguousarray(
        poseF.transpose(0, 2, 1).reshape(CD, NP)).astype(np.float16)    # [CD, NP]

    # ---- device kernel 1 -------------------------------------------------
    run1 = _get_runner("k1")
    in_maps = []
    for c in range(NCORES):
        l, r, rot = PM_ASSIGN[c]
        pr = poseFT[:, r * S:(r + 1) * S]
        if rot:
            pr = np.roll(pr, -ROT, axis=1)
        in_maps.append({
            "y": y[c * NPL:(c + 1) * NPL],
            "pose": pose_dev[c * NPL:(c + 1) * NPL],
            "pml": np.ascontiguousarray(poseFT[:, l * S:(l + 1) * S]),
            "pmr": np.ascontiguousarray(pr),
        })
    res1 = run1(in_maps)

    scores = _scores_from_out(res1)                                     # [NN]

    # ---- host: pm assembly + BCE ----------------------------------------
    pm = np.empty((NP, NP), np.float32)
    for c in range(6):
        l, r, rot = PM_ASSIGN[c]
        blk = res1[c]["pmo"].astype(np.float32)
        if rot:
            blk = np.roll(blk, ROT, axis=1)
        pm[l * S:(l + 1) * S, r * S:(r + 1) * S] = blk
        pm[r * S:(r + 1) * S, l * S:(l + 1) * S] = blk.T
    for r in range(NSTRIP):
        # top half rows 0:256 from the straight core, bottom rows 256:512
        # from the rotated core (un-roll its columns by +ROT)
        top = res1[r]["pmd"].astype(np.float32)            # [256, 512]
        bot = np.roll(res1[4 + r]["pmd"].astype(np.float32), ROT, axis=1)
        dblk = np.empty((S, S), np.float32)
        dblk[0:128, :] = top[0:128, :]
        dblk[128:256, 128:S] = top[128:256, 128:S]
        dblk[128:256, 0:128] = dblk[0:128, 128:256].T
        dblk[256:384, :] = bot[0:128, :]
        # rotated m1 chunk wrote cols 128:512 pre-roll -> valid post-roll
        # cols are [384:512) and [0:128)
        dblk[384:512, 384:S] = bot[128:256, 384:S]
        dblk[384:512, 0:128] = bot[128:256, 0:128]
        dblk[384:512, 128:384] = dblk[128:384, 384:512].T
        pm[r * S:(r + 1) * S, r * S:(r + 1) * S] = dblk

    p = (1.0 / (1.0 + np.exp(-scores))).astype(np.float32)
    lblf = nlab.astype(np.float32)
    loss_label = np.float32(
        np.mean(-(np.log(p) * lblf + np.log(1.0 - p) * (1.0 - lblf))))

    # ---- host: furthest selection ---------------------------------------
    ar = np.arange(NP)
    pm[ar, ar] = 1.0
    order = np.argsort(pm, axis=1, kind="stable")
    furthest = order[ar, ri]                                            # [NP]

    sg = scores.reshape(NP, K)
    lg = nlab.reshape(NP, K)
    maxp = np.maximum(np.max(np.where(lg == 0, sg, -np.inf), axis=1), -1.0)
    minp = np.minimum(np.min(np.where(lg == 1, sg, np.inf), axis=1), 1.0)

    # ---- device kernel 2: dots of gathered hard-positive rows ------------
    run2 = _get_runner("k2")
    y2 = y[furthest]                                                    # [NP, KCD]
    in_maps2 = []
    for c in range(NCORES):
        in_maps2.append({
            "y": y2[c * NPL:(c + 1) * NPL],
            "pose": pose_dev[c * NPL:(c + 1) * NPL],
        })
    res2 = run2(in_maps2)
    cur = _scores_from_out(res2).reshape(NP, K)

    nids = (furthest[:, None] * K + np.arange(K)).reshape(-1)           # [NN]
    lcur = nlab[nids].reshape(NP, K)
    maxcur = np.max(np.where(lcur == 1, cur, -np.inf), axis=1)
    maxp = np.maximum(maxp, maxcur)
    found = ~((maxp == -1.0) | (minp == 1.0))
    lt = np.where(found, maxp - minp + 2.0, 0.0).astype(np.float32)
    not_find = int(np.sum(~found))
    if not_find == NN:
        loss_triple = np.float32(0.0)
    else:
        loss_triple = np.float32(lt.sum(dtype=np.float32) / np.float32(NN - not_find))

    return (np.float32(loss_label), np.float32(loss_norm), np.float32(loss_triple))
